# revision 17
# baseline (speedup 1.0000x reference)
"""Two-layer GAT (PyG GATConv semantics) on 8 Trainium2 NeuronCores.

Strategy (graph/data parallel over destination nodes):
  - Nodes padded to NP=50176 = 8 * 6272; core c owns dst nodes
    [c*6272, (c+1)*6272), i.e. 98 blocks of M=64 dst nodes each.
  - Per layer, every core builds the feature-table rows for its own nodes
    (xp = x @ W plus attention-dot columns; bias folded into xp since
    softmax coefficients sum to 1), then an AllGather replicates the table.
  - Edges are bucketed by dst block on the host, sorted by src within each
    bucket (HBM locality), split into src<32768 / src>=32768 groups
    (dma_gather indices are int16) and padded to uniform GL/GH tiles of
    128 edges.  Per group of GRP=2 blocks the source rows are fetched with
    ONE multi-packet dma_gather per half (512B rows) from the replicated
    table.
  - a_dst per edge is computed ON-CHIP (no per-edge gather): phase A also
    produces the per-core a_dst node rows on partition 0 (tiny matmuls with
    the a_dst weight column as lhsT), broadcast across partitions
    (gpsimd.partition_broadcast), and per edge reduced against the
    destination onehot: adv = sum_m onehot[p,g,m] * adb[p, m].
  - Softmax-weighted aggregation is a matmul: lhsT = onehot(dstlocal) * w
    (bf16), rhs = gathered rows which carry literal 1.0 columns (folded
    into the bias row host-side) so the softmax denominator accumulates in
    the same PSUM tile; then out = u * (1/z).
  - exp without max-subtraction is safe here (|alpha| <~ 8).
"""

import os
import sys

for _p in ("/opt/trn_rl_repo", os.path.expanduser("~/.axon_site/_ro/trn_rl_repo")):
    if os.path.isdir(_p) and _p not in sys.path:
        sys.path.insert(0, _p)

import numpy as np
import ml_dtypes

import concourse.bass as bass
import concourse.bacc as bacc
import concourse.mybir as mybir
from concourse.tile import TileContext
from concourse.bass_utils import run_bass_kernel_spmd

BF16 = ml_dtypes.bfloat16

# problem constants (hardcoded per harness contract)
N = 50000
D_IN = 128
HID = 64
HEADS = 2
D_OUT = 64
SLOPE = 0.2

CORES = 8
P = 128          # edge tile size == matmul contraction == partitions
M = 64           # dst nodes per block
NP = 50176       # padded node count = CORES * NPC
NPC = NP // CORES        # 6272 nodes per core
BPC = NPC // M           # 98 blocks per core
NBLK = NP // M           # 784 blocks total
GRP = 2                  # blocks per gather group (98 = 49*2)
HALF = 32768             # int16 index range split

R1 = 256         # table1 row: [xp_h0(64)|1|1|xp_h1(64)|1|1|a_s(2)|a_d(2)|pad]
R2 = 128         # table2 row: [xp2(64)|1|1|a_s2|a_d2|pad]

# AllGather chunking: tables are laid out chunk-major ([chunk][core][local rows])
# so each chunk's AllGather output is contiguous; the first three chunks cover
# exactly local rows < 4096 on every core = table rows < 32768 (the int16 lo
# range of dma_gather indices).
CH_LEN = (1408, 1344, 1344, 1792, 384)
CH_CST = (0, 1408, 2752, 4096, 5888)        # local-row starts
CH_GST = (0, 11264, 22016, 32768, 47104)    # global table-row starts
NLOC = 4096                                  # local rows on the lo side


def _tau():
    """global permuted node id -> chunk-major table row."""
    t = np.zeros(NP, np.int64)
    for c in range(CORES):
        for k in range(len(CH_LEN)):
            r = np.arange(CH_CST[k], CH_CST[k] + CH_LEN[k])
            t[c * NPC + r] = CH_GST[k] + c * CH_LEN[k] + (r - CH_CST[k])
    return t


# ---------------------------------------------------------------- host prep

def _wrap16(v):
    """int16 index vector [n] -> dma_gather idx layout [128, n/16]."""
    w = v.reshape(-1, 16).T.astype(np.int16)      # [16, n/16]
    return np.ascontiguousarray(np.tile(w, (8, 1)))


def _balance_perm(src, dst):
    """Permute node ids so that per-block lo/hi edge counts are balanced
    (minimizes the uniform tile counts GL/GH).  Returns perm[orig] -> new."""
    lo_deg = np.bincount(dst[src < HALF], minlength=N).astype(np.float64)
    hi_deg = np.bincount(dst[src >= HALF], minlength=N).astype(np.float64)
    order = np.argsort(-(lo_deg + hi_deg), kind="stable")
    # nodes must stay on their side of the lo/hi boundary so the lo/hi edge
    # classification is invariant under the permutation; lo slots are the
    # first NLOC local rows of every core (chunk-major table rows < 32768)
    is_lo_blk = (np.arange(NBLK) % BPC) < (NLOC // M)
    lo_blocks = np.where(is_lo_blk)[0]
    hi_blocks = np.where(~is_lo_blk)[0]
    lo_b = np.zeros(NBLK)
    hi_b = np.zeros(NBLK)
    cnt_b = np.zeros(NBLK, np.int64)
    iL = NBLK / max(lo_deg.sum(), 1.0)
    iH = NBLK / max(hi_deg.sum(), 1.0)
    slot_of = np.zeros(N, np.int64)
    for n in order:
        cost = np.maximum((lo_b + lo_deg[n]) * iL, (hi_b + hi_deg[n]) * iH)
        cost[cnt_b >= M] = np.inf
        if n < HALF:
            b = int(lo_blocks[np.argmin(cost[lo_blocks])])
        else:
            b = int(hi_blocks[np.argmin(cost[hi_blocks])])
        slot_of[n] = b * M + cnt_b[b]
        lo_b[b] += lo_deg[n]
        hi_b[b] += hi_deg[n]
        cnt_b[b] += 1

    # swap-repair: push every block under the GL=ceil(mean_lo/P), GH caps
    TL = np.ceil(lo_b.mean() / P) * P
    TH = np.ceil(hi_b.mean() / P) * P
    blk_of = slot_of // M
    nodes_by_blk = [[] for _ in range(NBLK)]
    for n in range(N):
        nodes_by_blk[blk_of[n]].append(n)
    for _ in range(6000):
        viol = np.maximum(lo_b - TL, 0) + np.maximum(hi_b - TH, 0)
        b = int(np.argmax(viol))
        if viol[b] <= 0:
            break
        sideset = lo_blocks if is_lo_blk[b] else hi_blocks
        cand_b = sideset[np.argsort(np.maximum(lo_b[sideset] - TL, hi_b[sideset] - TH))[:24]]
        best = None
        for n1 in nodes_by_blk[b]:
            dl1, dh1 = lo_deg[n1], hi_deg[n1]
            for b2 in cand_b:
                if b2 == b:
                    continue
                for n2 in nodes_by_blk[b2]:
                    dl, dh = lo_deg[n2] - dl1, hi_deg[n2] - dh1
                    nv = (max(lo_b[b] + dl - TL, 0) + max(hi_b[b] + dh - TH, 0)
                          + max(lo_b[b2] - dl - TL, 0) + max(hi_b[b2] - dh - TH, 0))
                    if best is None or nv < best[0]:
                        best = (nv, n1, n2, b2)
            if best is not None and best[0] <= 0:
                break
        if best is None or best[0] >= viol[b] + max(lo_b[best[3]] - TL, 0) + max(hi_b[best[3]] - TH, 0):
            break
        _, n1, n2, b2 = best
        dl1, dh1 = lo_deg[n1], hi_deg[n1]
        dl2, dh2 = lo_deg[n2], hi_deg[n2]
        lo_b[b] += dl2 - dl1; hi_b[b] += dh2 - dh1
        lo_b[b2] += dl1 - dl2; hi_b[b2] += dh1 - dh2
        nodes_by_blk[b].remove(n1); nodes_by_blk[b].append(n2)
        nodes_by_blk[b2].remove(n2); nodes_by_blk[b2].append(n1)
        s1, s2 = slot_of[n1], slot_of[n2]
        slot_of[n1], slot_of[n2] = s2, s1
    return slot_of


def _edge_buckets(edge_index):
    """Bucket edges (with self loops) by 64-node dst block, sorted by src
    within each bucket, split into lo/hi src halves padded to uniform GL/GH
    tiles of 128 edges.  Node ids are permuted for lo/hi load balance."""
    ei = np.asarray(edge_index)
    src0 = np.concatenate([ei[0], np.arange(N, dtype=ei.dtype)]).astype(np.int64)
    dst0 = np.concatenate([ei[1], np.arange(N, dtype=ei.dtype)]).astype(np.int64)
    perm = _balance_perm(src0, dst0)
    src = perm[src0]
    dst = perm[dst0]
    tau = _tau()
    tab = tau[src]                       # chunk-major table row of each edge's src
    blk = dst // M
    half = (tab >= HALF).astype(np.int64)
    key = blk * 2 + half
    order = np.lexsort((tab, key))
    kcnt = np.bincount(key, minlength=NBLK * 2).reshape(NBLK, 2)
    GL = int(np.ceil(kcnt[:, 0].max() / P))
    GH = int(np.ceil(kcnt[:, 1].max() / P))
    G = GL + GH

    sk = key[order]
    starts = np.zeros(NBLK * 2 + 1, np.int64)
    starts[1:] = np.cumsum(kcnt.ravel())
    pos = np.arange(len(order)) - starts[sk]

    # per-block arrays [NBLK, G*128]; tiles 0..GL-1 lo, GL..G-1 hi
    srcl = np.zeros((NBLK, G * P), np.int64)
    dstloc = np.full((NBLK, G * P), 255.0, np.float32)
    b = blk[order]
    h = half[order]
    slot_pos = h * GL * P + pos
    srcl[b, slot_pos] = tab[order] - h * HALF
    dstloc[b, slot_pos] = (dst[order] % M).astype(np.float32)

    NGg = BPC // GRP
    per_core = []
    for c in range(CORES):
        s, e = c * BPC, (c + 1) * BPC
        sl = srcl[s:e].reshape(NGg, GRP, G * P)
        ilo = np.zeros((NGg, P, GRP * GL * 8), np.int16)
        ihi = np.zeros((NGg, P, max(GRP * GH * 8, 8)), np.int16)
        for g in range(NGg):
            lo = sl[g, :, : GL * P].reshape(-1)
            hi = sl[g, :, GL * P :].reshape(-1)
            ilo[g] = _wrap16(lo)
            if GH:
                ihi[g, :, : GRP * GH * 8] = _wrap16(hi)
        # dstpair bf16: per gather group [NG, 128, 2*T], SLOT-major
        # (slot = j*GL+g for lo, GRP*GL + j*GH + (g-GL) for hi), pairs (d, d)
        dloc_b = dstloc[s:e].reshape(BPC, G, P).transpose(0, 2, 1)   # [BPC, P, G]
        dg = dloc_b.reshape(NGg, GRP, P, G)
        lo2 = dg[:, :, :, :GL].transpose(0, 2, 1, 3).reshape(NGg, P, GRP * GL)
        hi2 = dg[:, :, :, GL:].transpose(0, 2, 1, 3).reshape(NGg, P, GRP * GH)
        slotd = np.concatenate([lo2, hi2], axis=2)                   # [NG, P, T]
        dstp = np.repeat(slotd[..., None], 2, axis=-1).reshape(NGg, P, 2 * GRP * G)
        per_core.append(
            dict(ilo=ilo, ihi=ihi, dstp=np.ascontiguousarray(dstp.astype(BF16)))
        )
    return GL, GH, per_core, perm


def _host_consts(x, W1, att_src1, att_dst1, b1, W2, att_src2, att_dst2, b2, perm):
    x = np.asarray(x, np.float32)
    W1 = np.asarray(W1, np.float32)
    W2 = np.asarray(W2, np.float32)
    b1 = np.asarray(b1, np.float32)
    b2 = np.asarray(b2, np.float32)
    a_s1 = np.asarray(att_src1, np.float32)
    a_d1 = np.asarray(att_dst1, np.float32)
    a_s2 = np.asarray(att_src2, np.float32)
    a_d2 = np.asarray(att_dst2, np.float32)

    # attention dots in input space: a_s[n,h] = x[n] @ col_h
    as1_cols = np.stack([W1[:, h * HID : (h + 1) * HID] @ a_s1[h] for h in range(HEADS)], 1)
    ad1_cols = np.stack([W1[:, h * HID : (h + 1) * HID] @ a_d1[h] for h in range(HEADS)], 1)
    as2_col = W2 @ a_s2[0]
    ad2_col = W2 @ a_d2[0]

    rhsA = np.zeros((P, 136), np.float32)
    rhsA[:, 0:64] = W1[:, 0:64]
    rhsA[:, 66:130] = W1[:, 64:128]
    rhsA[:, 132:134] = as1_cols
    rhsA[:, 134:136] = ad1_cols

    b1row = np.zeros((2 * M, 136), np.float32)
    b1row[:, 0:64] = b1[0:64]
    b1row[:, 66:130] = b1[64:128]
    b1row[:, 64:66] = 1.0      # softmax-denominator literal-1 columns
    b1row[:, 130:132] = 1.0

    rhsC = np.zeros((P, 68), np.float32)
    rhsC[:, 0:64] = W2
    rhsC[:, 66] = as2_col
    rhsC[:, 67] = ad2_col

    b2row = np.zeros((M, 68), np.float32)
    b2row[:, 0:64] = b2
    b2row[:, 64:66] = 1.0

    iota = np.tile(np.arange(64, dtype=np.float32), 2)
    iota_dup = np.broadcast_to(iota, (P, P)).copy().astype(BF16)
    ident = np.eye(P, dtype=np.float32).astype(BF16)

    xT = np.zeros((P, NP), np.float32)
    xT[:, perm] = x.T
    xT = xT.astype(BF16)

    return dict(
        rhsA=rhsA.astype(BF16), b1row=b1row,
        rhsC=rhsC.astype(BF16), b2row=b2row,
        iota=iota_dup, ident=ident, xT=xT,
    )


# ------------------------------------------------------------- bass program

def _ap(base, off, dims):
    """Custom strided view of a tile: keep partition dim, replace free dims."""
    a = base[:]
    return bass.AP(a.tensor, a.offset + off, [list(a.ap[0])] + [list(d) for d in dims])


def build_program(GL, GH, debug=False):
    G = GL + GH
    T = GRP * G                  # gather slots per group
    NLO = GRP * GL * P           # lo indices per group
    NHI = GRP * GH * P
    bf = mybir.dt.bfloat16
    f32 = mybir.dt.float32
    i16 = mybir.dt.int16
    EQ = mybir.AluOpType.is_equal
    MUL = mybir.AluOpType.mult
    ADD = mybir.AluOpType.add
    MAX = mybir.AluOpType.max
    Exp = mybir.ActivationFunctionType.Exp
    Copy = mybir.ActivationFunctionType.Copy

    def slot(j, g):
        return j * GL + g if g < GL else GRP * GL + j * GH + (g - GL)

    nc = bacc.Bacc(num_devices=CORES, num_swdge_queues=4)

    xT = nc.declare_dram_parameter("xT", [P, NPC], bf, isOutput=False)
    rhsA = nc.declare_dram_parameter("rhsA", [P, 136], bf, isOutput=False)
    b1row = nc.declare_dram_parameter("b1row", [2 * M, 136], f32, isOutput=False)
    rhsC = nc.declare_dram_parameter("rhsC", [P, 68], bf, isOutput=False)
    b2row = nc.declare_dram_parameter("b2row", [M, 68], f32, isOutput=False)
    iota = nc.declare_dram_parameter("iota", [P, P], bf, isOutput=False)
    ident = nc.declare_dram_parameter("ident", [P, P], bf, isOutput=False)
    NG = BPC // GRP
    ilo = nc.declare_dram_parameter("ilo", [NG, P, GRP * GL * 8], i16, isOutput=False)
    ihi = nc.declare_dram_parameter("ihi", [NG, P, max(GRP * GH * 8, 8)], i16, isOutput=False)
    dstp = nc.declare_dram_parameter("dstp", [NG, P, GRP * 2 * G], bf, isOutput=False)
    outp = nc.declare_dram_parameter("out", [NPC, D_OUT], f32, isOutput=True)

    if debug:
        dbg_ar1 = nc.declare_dram_parameter("dbg_ar1", [2, NPC], bf, isOutput=True)
        dbg_adb = nc.declare_dram_parameter("dbg_adb", [P, 256], bf, isOutput=True)
        dbg_adv = nc.declare_dram_parameter("dbg_adv", [P, 2 * GRP * (GL + GH)], f32, isOutput=True)
        dbg_alpha = nc.declare_dram_parameter("dbg_alpha", [P, 2 * GRP * (GL + GH)], f32, isOutput=True)
        dbg_w = nc.declare_dram_parameter("dbg_w", [P, 2 * GRP * (GL + GH)], f32, isOutput=True)
        dbg_oh = nc.declare_dram_parameter("dbg_oh", [P, GRP * (GL + GH) * 64], bf, isOutput=True)
        dbg_gt = nc.declare_dram_parameter("dbg_gt", [P, GRP * (GL + GH) * R1], bf, isOutput=True)
        dbg_ps = nc.declare_dram_parameter("dbg_ps", [M, 132], f32, isOutput=True)
        dbg_h = nc.declare_dram_parameter("dbg_h", [M, P], bf, isOutput=True)
        dbg_ar2 = nc.declare_dram_parameter("dbg_ar2", [1, NPC], bf, isOutput=True)

    t1l = nc.dram_tensor("t1l", [NPC, R1], bf)
    t1 = nc.dram_tensor("t1", [NP, R1], bf, addr_space="Shared")
    t2l = nc.dram_tensor("t2l", [NPC, R2], bf)
    t2 = nc.dram_tensor("t2", [NP, R2], bf, addr_space="Shared")

    groups = [list(range(CORES))]

    QN = [0]

    def next_q():
        QN[0] = (QN[0] + 1) % 4
        return QN[0]

    def chunked_gather(out_tile, slot0, tab_ap, idx_tile, nslots, elem):
        CH = 8  # 1024 indices per dma_gather (single-packet 64-desc limit)
        done = 0
        while done < nslots:
            take = min(CH, nslots - done)
            n = take * P
            nc.gpsimd.dma_gather(
                _ap(out_tile, (slot0 + done) * elem, [[elem, take], [1, elem]]),
                tab_ap,
                idx_tile[:, done * 8 : (done + take) * 8],
                n, n, elem, queue_num=next_q(),
            )
            done += take

    with TileContext(nc) as tc:
        with tc.tile_pool(name="const", bufs=1) as cp:
            rhsA_sb = cp.tile([P, 136], bf)
            nc.sync.dma_start(out=rhsA_sb[:], in_=rhsA[:])
            b1row_sb = cp.tile([2 * M, 136], f32)
            nc.sync.dma_start(out=b1row_sb[:], in_=b1row[:])
            rhsC_sb = cp.tile([P, 68], bf)
            nc.sync.dma_start(out=rhsC_sb[:], in_=rhsC[:])
            b2row_sb = cp.tile([M, 68], f32)
            nc.sync.dma_start(out=b2row_sb[:], in_=b2row[:])
            iota_sb = cp.tile([P, P], bf)
            nc.sync.dma_start(out=iota_sb[:], in_=iota[:])
            ident_sb = cp.tile([P, P], bf)
            nc.sync.dma_start(out=ident_sb[:], in_=ident[:])
            zero_sb = cp.tile([M, P], f32)
            nc.vector.memset(zero_sb[:], 0.0)
            adb1 = cp.tile([P, 2 * NPC], bf)     # a_dst layer1: head h at cols h*NPC + n
            adb2 = cp.tile([P, NPC], bf)         # a_dst layer2
            ar2 = cp.tile([1, NPC], bf)

            # ---------------- phase A: layer-1 table rows for own nodes
            with (
                tc.tile_pool(name="sbA0", bufs=1) as pa0,
                tc.tile_pool(name="sbA", bufs=3) as pa,
                tc.tile_pool(name="psA", bufs=2, space="PSUM") as ppa,
            ):
                xT_sb = pa0.tile([P, NPC], bf)
                nc.sync.dma_start(out=xT_sb[:], in_=xT[:])
                ar1_h0 = pa0.tile([1, NPC], bf)
                ar1_h1 = pa0.tile([1, NPC], bf)
                ag1_chunks = {}
                for k in range(len(CH_LEN)):
                    eb = (CH_CST[k] + CH_LEN[k]) // M - 1
                    ag1_chunks[eb - (eb % 2)] = k
                for b in range(0, BPC, 2):
                    ps = ppa.tile([2 * M, 136], f32, tag="psA")
                    nc.tensor.matmul(
                        ps[:], lhsT=xT_sb[:, b * M : (b + 2) * M], rhs=rhsA_sb[:],
                        start=True, stop=True,
                    )
                    row = pa.tile([2 * M, R1], bf, tag="rowA")
                    nc.vector.tensor_tensor(out=row[:, 0:136], in0=ps[:], in1=b1row_sb[:], op=ADD)
                    nc.sync.dma_start(out=t1l[b * M : (b + 2) * M, :], in_=row[:])
                    if b in ag1_chunks:
                        k = ag1_chunks[b]
                        r0 = CH_CST[k]
                        g0 = CH_GST[k]
                        ln = CH_LEN[k]
                        nc.gpsimd.collective_compute(
                            "AllGather", mybir.AluOpType.bypass, replica_groups=groups,
                            ins=[t1l[r0 : r0 + ln, :]],
                            outs=[t1[g0 : g0 + CORES * ln, :]],
                        )
                # a_dst rows on partition 0: 512-node chunks per head
                off = 0
                while off < NPC:
                    w_ = min(512, NPC - off)
                    for hh, art in ((0, ar1_h0), (1, ar1_h1)):
                        psr = ppa.tile([1, 512], f32, tag=f"psr{hh}")
                        nc.tensor.matmul(
                            psr[:, 0:w_], lhsT=rhsA_sb[:, 134 + hh : 135 + hh],
                            rhs=xT_sb[:, off : off + w_], start=True, stop=True,
                        )
                        nc.vector.tensor_copy(out=art[:, off : off + w_], in_=psr[:, 0:w_])
                    off += w_

                nc.gpsimd.partition_broadcast(adb1[:, 0:NPC], ar1_h0[:], P)
                nc.gpsimd.partition_broadcast(adb1[:, NPC : 2 * NPC], ar1_h1[:], P)
                if debug:
                    nc.sync.dma_start(out=dbg_ar1[0:1, :], in_=ar1_h0[:])
                    nc.sync.dma_start(out=dbg_ar1[1:2, :], in_=ar1_h1[:])
                    nc.sync.dma_start(out=dbg_adb[:, 0:128], in_=adb1[:, 0:128])
                    nc.sync.dma_start(out=dbg_adb[:, 128:256], in_=adb1[:, NPC : NPC + 128])


            # ---------------- phase B: layer-1 edges -> h -> table2 rows
            with (
                tc.tile_pool(name="sbB", bufs=3) as pb,
                tc.tile_pool(name="psB", bufs=2, space="PSUM") as ppb,
                tc.tile_pool(name="psB1", bufs=1, space="PSUM") as ppb1,
            ):
                for s in range(NG):
                    li = pb.tile([P, GRP * GL * 8], i16, tag="li")
                    nc.sync.dma_start(out=li[:], in_=ilo[s])
                    hi_t = pb.tile([P, max(GRP * GH * 8, 8)], i16, tag="hi")
                    nc.sync.dma_start(out=hi_t[:], in_=ihi[s])
                    dp = pb.tile([P, GRP * 2 * G], bf, tag="dp")
                    nc.sync.dma_start(out=dp[:], in_=dstp[s])

                    gt = pb.tile([P, T * R1], bf, tag="gt")
                    chunked_gather(gt, 0, t1[:], li, GRP * GL, R1)
                    if GH:
                        chunked_gather(gt, GRP * GL, t1[HALF:, :], hi_t, GRP * GH, R1)

                    # onehot, SLOT-major single copy: cols slot*64 + m
                    oh = pb.tile([P, T * 64], bf, tag="oh")
                    nc.vector.tensor_tensor(
                        out=_ap(oh, 0, [[64, T], [2, 32], [1, 2]]),
                        in0=_ap(iota_sb, 0, [[0, T], [2, 32], [1, 2]]),
                        in1=_ap(dp, 0, [[2, T], [0, 32], [1, 2]]),
                        op=EQ,
                    )
                    # per-edge a_dst: tmp[slot,hh,m] = oh[slot,m] * adb1[hh,node],
                    # then one reduce over m for all slots
                    tmp = pb.tile([P, T * P], bf, tag="tmp")
                    for j in range(GRP):
                        nd = (s * GRP + j) * M
                        nc.vector.tensor_tensor(
                            out=_ap(tmp, j * GL * P, [[P, GL], [64, 2], [1, 64]]),
                            in0=_ap(oh, j * GL * 64, [[64, GL], [0, 2], [1, 64]]),
                            in1=_ap(adb1, nd, [[0, GL], [NPC, 2], [1, 64]]),
                            op=MUL,
                        )
                        if GH:
                            nc.vector.tensor_tensor(
                                out=_ap(tmp, (GRP * GL + j * GH) * P, [[P, GH], [64, 2], [1, 64]]),
                                in0=_ap(oh, (GRP * GL + j * GH) * 64, [[64, GH], [0, 2], [1, 64]]),
                                in1=_ap(adb1, nd, [[0, GH], [NPC, 2], [1, 64]]),
                                op=MUL,
                            )
                    adv = pb.tile([P, T * 2], f32, tag="adv")
                    nc.vector.tensor_reduce(
                        out=adv[:],
                        in_=_ap(tmp, 0, [[P, T], [64, 2], [1, 64]]),
                        axis=mybir.AxisListType.X, op=ADD,
                    )
                    # alpha = a_src[src] + a_dst[dst]; w = exp(lrelu(alpha))
                    alpha = pb.tile([P, T * 2], f32, tag="alpha")
                    nc.vector.tensor_tensor(
                        out=alpha[:],
                        in0=_ap(gt, 132, [[R1, T], [1, 2]]),
                        in1=adv[:], op=ADD,
                    )
                    lr = pb.tile([P, T * 2], f32, tag="lr")
                    nc.vector.scalar_tensor_tensor(
                        out=lr[:], in0=alpha[:], scalar=SLOPE, in1=alpha[:],
                        op0=MUL, op1=MAX,
                    )
                    w = pb.tile([P, T * 2], f32, tag="w")
                    nc.scalar.activation(w[:], lr[:], Exp)
                    if debug and s == 0:
                        nc.sync.dma_start(out=dbg_adv[:], in_=adv[:])
                        nc.sync.dma_start(out=dbg_alpha[:], in_=alpha[:])
                        nc.sync.dma_start(out=dbg_w[:], in_=w[:])
                        nc.sync.dma_start(out=dbg_oh[:], in_=oh[:])
                        nc.sync.dma_start(out=dbg_gt[:], in_=gt[:])

                    # ow = oh * w (stride-0 broadcast of w over the 64 m cols);
                    # reuses tmp's buffer (tmp is dead after the reduce)
                    ow = tmp
                    nc.vector.tensor_tensor(
                        out=_ap(ow, 0, [[P, T], [64, 2], [1, 64]]),
                        in0=_ap(oh, 0, [[64, T], [0, 2], [1, 64]]),
                        in1=_ap(w, 0, [[2, T], [1, 2], [0, 64]]),
                        op=MUL,
                    )

                    for j in range(GRP):
                        blk = s * GRP + j
                        ps0 = ppb.tile([M, 66], f32, tag="psB0")
                        ps1 = ppb.tile([M, 66], f32, tag="psB1")
                        for g in range(G):
                            base = slot(j, g) * R1
                            cb = slot(j, g) * P
                            nc.tensor.matmul(
                                ps0[:],
                                lhsT=ow[:, cb : cb + 64],
                                rhs=gt[:, base : base + 66],
                                start=(g == 0), stop=(g == G - 1),
                            )
                            nc.tensor.matmul(
                                ps1[:],
                                lhsT=ow[:, cb + 64 : cb + 128],
                                rhs=gt[:, base + 66 : base + 132],
                                start=(g == 0), stop=(g == G - 1),
                            )
                        zs = pb.tile([M, 2], f32, tag="zs")
                        nc.vector.tensor_scalar(zs[:, 0:1], ps0[:, 64:65], 1e-30, None, MAX)
                        nc.vector.tensor_scalar(zs[:, 1:2], ps1[:, 64:65], 1e-30, None, MAX)
                        rz = pb.tile([M, 2], f32, tag="rz")
                        nc.vector.reciprocal(rz[:], zs[:])
                        h = pb.tile([M, P], bf, tag="h")
                        nc.vector.scalar_tensor_tensor(
                            out=h[:, 0:64], in0=ps0[:, 0:64], scalar=rz[:, 0:1],
                            in1=zero_sb[:, 0:64], op0=MUL, op1=MAX,
                        )
                        nc.vector.scalar_tensor_tensor(
                            out=h[:, 64:128], in0=ps1[:, 0:64], scalar=rz[:, 1:2],
                            in1=zero_sb[:, 64:128], op0=MUL, op1=MAX,
                        )
                        if debug and s == 0 and j == 0:
                            pscopy = pb.tile([M, 132], f32, tag="pscopy")
                            nc.vector.tensor_copy(out=pscopy[:, 0:66], in_=ps0[:])
                            nc.vector.tensor_copy(out=pscopy[:, 66:132], in_=ps1[:])
                            nc.sync.dma_start(out=dbg_ps[:], in_=pscopy[:])
                            nc.sync.dma_start(out=dbg_h[:], in_=h[:])
                        pt = ppb1.tile([P, M], bf, tag="psT")
                        nc.tensor.transpose(out=pt[:], in_=h[:], identity=ident_sb[:64, :64])
                        hT = pb.tile([P, M], bf, tag="hT")
                        nc.scalar.activation(hT[:], pt[:], Copy)
                        psc = ppb.tile([M, 68], f32, tag="psC")
                        nc.tensor.matmul(psc[:], lhsT=hT[:], rhs=rhsC_sb[:], start=True, stop=True)
                        row2 = pb.tile([M, R2], bf, tag="row2")
                        nc.vector.tensor_tensor(out=row2[:, 0:68], in0=psc[:], in1=b2row_sb[:], op=ADD)
                        nc.scalar.dma_start(out=t2l[blk * M : (blk + 1) * M, :], in_=row2[:])
                        # layer-2 a_dst row chunk (nodes of this block on partition 0)
                        psd = ppb1.tile([1, M], f32, tag="psD2")
                        nc.tensor.matmul(psd[:], lhsT=rhsC_sb[:, 67:68], rhs=hT[:, 0:M],
                                         start=True, stop=True)
                        nc.scalar.activation(ar2[:, blk * M : (blk + 1) * M], psd[:], Copy)
                    ag2_chunks = {
                        ((CH_CST[k] + CH_LEN[k]) // M - 1) // GRP: k
                        for k in range(len(CH_LEN))
                    }
                    if s in ag2_chunks:
                        k = ag2_chunks[s]
                        r0 = CH_CST[k]
                        g0 = CH_GST[k]
                        ln = CH_LEN[k]
                        nc.gpsimd.collective_compute(
                            "AllGather", mybir.AluOpType.bypass, replica_groups=groups,
                            ins=[t2l[r0 : r0 + ln, :]],
                            outs=[t2[g0 : g0 + CORES * ln, :]],
                        )

            nc.gpsimd.partition_broadcast(adb2[:], ar2[:], P)
            if debug:
                nc.sync.dma_start(out=dbg_ar2[:], in_=ar2[:])


            # ---------------- phase D: layer-2 edges -> output
            with (
                tc.tile_pool(name="sbD", bufs=4) as pd,
                tc.tile_pool(name="psD", bufs=2, space="PSUM") as ppd,
            ):
                for s in range(NG):
                    li = pd.tile([P, GRP * GL * 8], i16, tag="li2")
                    nc.sync.dma_start(out=li[:], in_=ilo[s])
                    hi_t = pd.tile([P, max(GRP * GH * 8, 8)], i16, tag="hi2")
                    nc.sync.dma_start(out=hi_t[:], in_=ihi[s])
                    dp = pd.tile([P, GRP * 2 * G], bf, tag="dp2")
                    nc.sync.dma_start(out=dp[:], in_=dstp[s])

                    gt2 = pd.tile([P, T * R2], bf, tag="gt2")
                    chunked_gather(gt2, 0, t2[:], li, GRP * GL, R2)
                    if GH:
                        chunked_gather(gt2, GRP * GL, t2[HALF:, :], hi_t, GRP * GH, R2)

                    oh = pd.tile([P, T * 64], bf, tag="oh2")
                    nc.vector.tensor_tensor(
                        out=_ap(oh, 0, [[64, T], [2, 32], [1, 2]]),
                        in0=_ap(iota_sb, 0, [[0, T], [2, 32], [1, 2]]),
                        in1=_ap(dp, 0, [[2, T], [0, 32], [1, 2]]),
                        op=EQ,
                    )
                    tmp = pd.tile([P, T * 64], bf, tag="tmp2")
                    for j in range(GRP):
                        nd = (s * GRP + j) * M
                        nc.vector.tensor_tensor(
                            out=_ap(tmp, j * GL * 64, [[64, GL], [1, 64]]),
                            in0=_ap(oh, j * GL * 64, [[64, GL], [1, 64]]),
                            in1=_ap(adb2, nd, [[0, GL], [1, 64]]),
                            op=MUL,
                        )
                        if GH:
                            hb = (GRP * GL + j * GH) * 64
                            nc.vector.tensor_tensor(
                                out=_ap(tmp, hb, [[64, GH], [1, 64]]),
                                in0=_ap(oh, hb, [[64, GH], [1, 64]]),
                                in1=_ap(adb2, nd, [[0, GH], [1, 64]]),
                                op=MUL,
                            )
                    adv = pd.tile([P, T], f32, tag="adv2")
                    nc.vector.tensor_reduce(
                        out=adv[:],
                        in_=_ap(tmp, 0, [[64, T], [1, 64]]),
                        axis=mybir.AxisListType.X, op=ADD,
                    )
                    alpha = pd.tile([P, T], f32, tag="alpha2")
                    nc.vector.tensor_tensor(
                        out=alpha[:],
                        in0=_ap(gt2, 66, [[R2, T]]),
                        in1=adv[:], op=ADD,
                    )
                    lr = pd.tile([P, T], f32, tag="lr2")
                    nc.vector.scalar_tensor_tensor(
                        out=lr[:], in0=alpha[:], scalar=SLOPE, in1=alpha[:],
                        op0=MUL, op1=MAX,
                    )
                    w = pd.tile([P, T], f32, tag="w2")
                    nc.scalar.activation(w[:], lr[:], Exp)

                    ow = tmp
                    nc.vector.tensor_tensor(
                        out=_ap(ow, 0, [[64, T], [1, 64]]),
                        in0=_ap(oh, 0, [[64, T], [1, 64]]),
                        in1=_ap(w, 0, [[1, T], [0, 64]]),
                        op=MUL,
                    )

                    for j in range(GRP):
                        blk = s * GRP + j
                        ps = ppd.tile([M, 66], f32, tag="psD")
                        for g in range(G):
                            base = slot(j, g) * R2
                            cb = slot(j, g) * 64
                            nc.tensor.matmul(
                                ps[:],
                                lhsT=ow[:, cb : cb + 64],
                                rhs=gt2[:, base : base + 66],
                                start=(g == 0), stop=(g == G - 1),
                            )
                        zs = pd.tile([M, 1], f32, tag="zs2")
                        nc.vector.tensor_scalar(zs[:], ps[:, 64:65], 1e-30, None, MAX)
                        rz = pd.tile([M, 1], f32, tag="rz2")
                        nc.vector.reciprocal(rz[:], zs[:])
                        o2 = pd.tile([M, D_OUT], f32, tag="o2")
                        nc.vector.scalar_tensor_tensor(
                            out=o2[:], in0=ps[:, 0:64], scalar=rz[:, 0:1],
                            in1=zero_sb[:, 0:64], op0=MUL, op1=ADD,
                        )
                        nc.scalar.dma_start(out=outp[blk * M : (blk + 1) * M, :], in_=o2[:])

    nc.finalize()
    return nc


# ------------------------------------------------------------------ driver

_CACHE = {}


def kernel(x, edge_index, W1, att_src1, att_dst1, b1, W2, att_src2, att_dst2, b2):
    GL, GH, per_core, perm = _edge_buckets(edge_index)
    consts = _host_consts(x, W1, att_src1, att_dst1, b1, W2, att_src2, att_dst2, b2, perm)

    if (GL, GH) not in _CACHE:
        _CACHE[(GL, GH)] = build_program(GL, GH)
    nc = _CACHE[(GL, GH)]

    in_maps = []
    for c in range(CORES):
        m = dict(consts)
        m["xT"] = np.ascontiguousarray(consts["xT"][:, c * NPC : (c + 1) * NPC])
        m.update(per_core[c])
        in_maps.append(m)

    res = run_bass_kernel_spmd(nc, in_maps, list(range(CORES)))
    out = np.concatenate([np.asarray(res.results[c]["out"]) for c in range(CORES)], axis=0)
    return np.ascontiguousarray(out[perm]).astype(np.float32)


# revision 18
# speedup vs baseline: 1.1872x; 1.1872x over previous
"""Two-layer GAT (PyG GATConv semantics) on 8 Trainium2 NeuronCores.

Strategy (graph/data parallel over destination nodes):
  - Nodes padded to NP=50176 = 8 * 6272; core c owns dst nodes
    [c*6272, (c+1)*6272), i.e. 98 blocks of M=64 dst nodes each.
  - Per layer, every core builds the feature-table rows for its own nodes
    (xp = x @ W plus attention-dot columns; bias folded into xp since
    softmax coefficients sum to 1), then an AllGather replicates the table.
  - Edges are bucketed by dst block on the host, sorted by src within each
    bucket (HBM locality), split into src<32768 / src>=32768 groups
    (dma_gather indices are int16) and padded to uniform GL/GH tiles of
    128 edges.  Per group of GRP=2 blocks the source rows are fetched with
    ONE multi-packet dma_gather per half (512B rows) from the replicated
    table.
  - a_dst per edge is computed ON-CHIP (no per-edge gather): phase A also
    produces the per-core a_dst node rows on partition 0 (tiny matmuls with
    the a_dst weight column as lhsT), broadcast across partitions
    (gpsimd.partition_broadcast), and per edge reduced against the
    destination onehot: adv = sum_m onehot[p,g,m] * adb[p, m].
  - Softmax-weighted aggregation is a matmul: lhsT = onehot(dstlocal) * w
    (bf16), rhs = gathered rows which carry literal 1.0 columns (folded
    into the bias row host-side) so the softmax denominator accumulates in
    the same PSUM tile; then out = u * (1/z).
  - exp without max-subtraction is safe here (|alpha| <~ 8).
"""

import os
import sys

for _p in ("/opt/trn_rl_repo", os.path.expanduser("~/.axon_site/_ro/trn_rl_repo")):
    if os.path.isdir(_p) and _p not in sys.path:
        sys.path.insert(0, _p)

import numpy as np
import ml_dtypes

import concourse.bass as bass
import concourse.bacc as bacc
import concourse.mybir as mybir
from concourse.tile import TileContext
from concourse.bass_utils import run_bass_kernel_spmd

BF16 = ml_dtypes.bfloat16

# problem constants (hardcoded per harness contract)
N = 50000
D_IN = 128
HID = 64
HEADS = 2
D_OUT = 64
SLOPE = 0.2

CORES = 8
P = 128          # edge tile size == matmul contraction == partitions
M = 64           # dst nodes per block
NP = 50176       # padded node count = CORES * NPC
NPC = NP // CORES        # 6272 nodes per core
BPC = NPC // M           # 98 blocks per core
NBLK = NP // M           # 784 blocks total
GRP = 2                  # blocks per gather group (98 = 49*2)
HALF = 32768             # int16 index range split

R1 = 256         # table1 row: [xp_h0(64)|1|1|xp_h1(64)|1|1|a_s(2)|a_d(2)|pad]
R2 = 128         # table2 row: [xp2(64)|1|1|a_s2|a_d2|pad]

# AllGather chunking: tables are laid out chunk-major ([chunk][core][local rows])
# so each chunk's AllGather output is contiguous; the first three chunks cover
# exactly local rows < 4096 on every core = table rows < 32768 (the int16 lo
# range of dma_gather indices).
CH_LEN = (1408, 1344, 1344, 1344, 832)
CH_CST = (0, 1408, 2752, 4096, 5440)        # local-row starts
CH_GST = (0, 11264, 22016, 32768, 43520)    # global table-row starts
NLOC = 4096                                  # local rows on the lo side


def _tau():
    """global permuted node id -> chunk-major table row."""
    t = np.zeros(NP, np.int64)
    for c in range(CORES):
        for k in range(len(CH_LEN)):
            r = np.arange(CH_CST[k], CH_CST[k] + CH_LEN[k])
            t[c * NPC + r] = CH_GST[k] + c * CH_LEN[k] + (r - CH_CST[k])
    return t


# ---------------------------------------------------------------- host prep

def _wrap16(v):
    """int16 index vector [n] -> dma_gather idx layout [128, n/16]."""
    w = v.reshape(-1, 16).T.astype(np.int16)      # [16, n/16]
    return np.ascontiguousarray(np.tile(w, (8, 1)))


def _balance_perm(src, dst):
    """Permute node ids so that per-block lo/hi edge counts are balanced
    (minimizes the uniform tile counts GL/GH).  Returns perm[orig] -> new."""
    lo_deg = np.bincount(dst[src < HALF], minlength=N).astype(np.float64)
    hi_deg = np.bincount(dst[src >= HALF], minlength=N).astype(np.float64)
    order = np.argsort(-(lo_deg + hi_deg), kind="stable")
    # nodes must stay on their side of the lo/hi boundary so the lo/hi edge
    # classification is invariant under the permutation; lo slots are the
    # first NLOC local rows of every core (chunk-major table rows < 32768)
    is_lo_blk = (np.arange(NBLK) % BPC) < (NLOC // M)
    lo_blocks = np.where(is_lo_blk)[0]
    hi_blocks = np.where(~is_lo_blk)[0]
    lo_b = np.zeros(NBLK)
    hi_b = np.zeros(NBLK)
    cnt_b = np.zeros(NBLK, np.int64)
    iL = NBLK / max(lo_deg.sum(), 1.0)
    iH = NBLK / max(hi_deg.sum(), 1.0)
    slot_of = np.zeros(N, np.int64)
    for n in order:
        cost = np.maximum((lo_b + lo_deg[n]) * iL, (hi_b + hi_deg[n]) * iH)
        cost[cnt_b >= M] = np.inf
        if n < HALF:
            b = int(lo_blocks[np.argmin(cost[lo_blocks])])
        else:
            b = int(hi_blocks[np.argmin(cost[hi_blocks])])
        slot_of[n] = b * M + cnt_b[b]
        lo_b[b] += lo_deg[n]
        hi_b[b] += hi_deg[n]
        cnt_b[b] += 1

    # swap-repair: push every block under the GL=ceil(mean_lo/P), GH caps
    TL = np.ceil(lo_b.mean() / P) * P
    TH = np.ceil(hi_b.mean() / P) * P
    blk_of = slot_of // M
    nodes_by_blk = [[] for _ in range(NBLK)]
    for n in range(N):
        nodes_by_blk[blk_of[n]].append(n)
    for _ in range(6000):
        viol = np.maximum(lo_b - TL, 0) + np.maximum(hi_b - TH, 0)
        b = int(np.argmax(viol))
        if viol[b] <= 0:
            break
        sideset = lo_blocks if is_lo_blk[b] else hi_blocks
        cand_b = sideset[np.argsort(np.maximum(lo_b[sideset] - TL, hi_b[sideset] - TH))[:24]]
        best = None
        for n1 in nodes_by_blk[b]:
            dl1, dh1 = lo_deg[n1], hi_deg[n1]
            for b2 in cand_b:
                if b2 == b:
                    continue
                for n2 in nodes_by_blk[b2]:
                    dl, dh = lo_deg[n2] - dl1, hi_deg[n2] - dh1
                    nv = (max(lo_b[b] + dl - TL, 0) + max(hi_b[b] + dh - TH, 0)
                          + max(lo_b[b2] - dl - TL, 0) + max(hi_b[b2] - dh - TH, 0))
                    if best is None or nv < best[0]:
                        best = (nv, n1, n2, b2)
            if best is not None and best[0] <= 0:
                break
        if best is None or best[0] >= viol[b] + max(lo_b[best[3]] - TL, 0) + max(hi_b[best[3]] - TH, 0):
            break
        _, n1, n2, b2 = best
        dl1, dh1 = lo_deg[n1], hi_deg[n1]
        dl2, dh2 = lo_deg[n2], hi_deg[n2]
        lo_b[b] += dl2 - dl1; hi_b[b] += dh2 - dh1
        lo_b[b2] += dl1 - dl2; hi_b[b2] += dh1 - dh2
        nodes_by_blk[b].remove(n1); nodes_by_blk[b].append(n2)
        nodes_by_blk[b2].remove(n2); nodes_by_blk[b2].append(n1)
        s1, s2 = slot_of[n1], slot_of[n2]
        slot_of[n1], slot_of[n2] = s2, s1
    return slot_of


def _edge_buckets(edge_index):
    """Bucket edges (with self loops) by 64-node dst block, sorted by src
    within each bucket, split into lo/hi src halves padded to uniform GL/GH
    tiles of 128 edges.  Node ids are permuted for lo/hi load balance."""
    ei = np.asarray(edge_index)
    src0 = np.concatenate([ei[0], np.arange(N, dtype=ei.dtype)]).astype(np.int64)
    dst0 = np.concatenate([ei[1], np.arange(N, dtype=ei.dtype)]).astype(np.int64)
    perm = _balance_perm(src0, dst0)
    src = perm[src0]
    dst = perm[dst0]
    tau = _tau()
    tab = tau[src]                       # chunk-major table row of each edge's src
    blk = dst // M
    half = (tab >= HALF).astype(np.int64)
    key = blk * 2 + half
    order = np.lexsort((tab, key))
    kcnt = np.bincount(key, minlength=NBLK * 2).reshape(NBLK, 2)
    GL = int(np.ceil(kcnt[:, 0].max() / P))
    GH = int(np.ceil(kcnt[:, 1].max() / P))
    G = GL + GH

    sk = key[order]
    starts = np.zeros(NBLK * 2 + 1, np.int64)
    starts[1:] = np.cumsum(kcnt.ravel())
    pos = np.arange(len(order)) - starts[sk]

    # per-block arrays [NBLK, G*128]; tiles 0..GL-1 lo, GL..G-1 hi
    srcl = np.zeros((NBLK, G * P), np.int64)
    dstloc = np.full((NBLK, G * P), 255.0, np.float32)
    b = blk[order]
    h = half[order]
    slot_pos = h * GL * P + pos
    srcl[b, slot_pos] = tab[order] - h * HALF
    dstloc[b, slot_pos] = (dst[order] % M).astype(np.float32)

    NGg = BPC // GRP
    per_core = []
    for c in range(CORES):
        s, e = c * BPC, (c + 1) * BPC
        sl = srcl[s:e].reshape(NGg, GRP, G * P)
        ilo = np.zeros((NGg, P, GRP * GL * 8), np.int16)
        ihi = np.zeros((NGg, P, max(GRP * GH * 8, 8)), np.int16)
        for g in range(NGg):
            lo = sl[g, :, : GL * P].reshape(-1)
            hi = sl[g, :, GL * P :].reshape(-1)
            ilo[g] = _wrap16(lo)
            if GH:
                ihi[g, :, : GRP * GH * 8] = _wrap16(hi)
        # dstpair bf16: per gather group [NG, 128, 2*T], SLOT-major
        # (slot = j*GL+g for lo, GRP*GL + j*GH + (g-GL) for hi), pairs (d, d)
        dloc_b = dstloc[s:e].reshape(BPC, G, P).transpose(0, 2, 1)   # [BPC, P, G]
        dg = dloc_b.reshape(NGg, GRP, P, G)
        lo2 = dg[:, :, :, :GL].transpose(0, 2, 1, 3).reshape(NGg, P, GRP * GL)
        hi2 = dg[:, :, :, GL:].transpose(0, 2, 1, 3).reshape(NGg, P, GRP * GH)
        slotd = np.concatenate([lo2, hi2], axis=2)                   # [NG, P, T]
        dstp = np.repeat(slotd[..., None], 2, axis=-1).reshape(NGg, P, 2 * GRP * G)
        per_core.append(
            dict(ilo=ilo, ihi=ihi, dstp=np.ascontiguousarray(dstp.astype(BF16)))
        )
    return GL, GH, per_core, perm


def _host_consts(x, W1, att_src1, att_dst1, b1, W2, att_src2, att_dst2, b2, perm):
    x = np.asarray(x, np.float32)
    W1 = np.asarray(W1, np.float32)
    W2 = np.asarray(W2, np.float32)
    b1 = np.asarray(b1, np.float32)
    b2 = np.asarray(b2, np.float32)
    a_s1 = np.asarray(att_src1, np.float32)
    a_d1 = np.asarray(att_dst1, np.float32)
    a_s2 = np.asarray(att_src2, np.float32)
    a_d2 = np.asarray(att_dst2, np.float32)

    # attention dots in input space: a_s[n,h] = x[n] @ col_h
    as1_cols = np.stack([W1[:, h * HID : (h + 1) * HID] @ a_s1[h] for h in range(HEADS)], 1)
    ad1_cols = np.stack([W1[:, h * HID : (h + 1) * HID] @ a_d1[h] for h in range(HEADS)], 1)
    as2_col = W2 @ a_s2[0]
    ad2_col = W2 @ a_d2[0]

    rhsA = np.zeros((P, 136), np.float32)
    rhsA[:, 0:64] = W1[:, 0:64]
    rhsA[:, 66:130] = W1[:, 64:128]
    rhsA[:, 132:134] = as1_cols
    rhsA[:, 134:136] = ad1_cols

    b1row = np.zeros((2 * M, 136), np.float32)
    b1row[:, 0:64] = b1[0:64]
    b1row[:, 66:130] = b1[64:128]
    b1row[:, 64:66] = 1.0      # softmax-denominator literal-1 columns
    b1row[:, 130:132] = 1.0

    rhsC = np.zeros((P, 68), np.float32)
    rhsC[:, 0:64] = W2
    rhsC[:, 66] = as2_col
    rhsC[:, 67] = ad2_col

    b2row = np.zeros((M, 68), np.float32)
    b2row[:, 0:64] = b2
    b2row[:, 64:66] = 1.0

    iota = np.tile(np.arange(64, dtype=np.float32), 2)
    iota_dup = np.broadcast_to(iota, (P, P)).copy().astype(BF16)
    ident = np.eye(P, dtype=np.float32).astype(BF16)

    xT = np.zeros((P, NP), np.float32)
    xT[:, perm] = x.T
    xT = xT.astype(BF16)

    return dict(
        rhsA=rhsA.astype(BF16), b1row=b1row,
        rhsC=rhsC.astype(BF16), b2row=b2row,
        iota=iota_dup, ident=ident, xT=xT,
    )


# ------------------------------------------------------------- bass program

def _ap(base, off, dims):
    """Custom strided view of a tile: keep partition dim, replace free dims."""
    a = base[:]
    return bass.AP(a.tensor, a.offset + off, [list(a.ap[0])] + [list(d) for d in dims])


def build_program(GL, GH, debug=False):
    G = GL + GH
    T = GRP * G                  # gather slots per group
    NLO = GRP * GL * P           # lo indices per group
    NHI = GRP * GH * P
    bf = mybir.dt.bfloat16
    f32 = mybir.dt.float32
    i16 = mybir.dt.int16
    EQ = mybir.AluOpType.is_equal
    MUL = mybir.AluOpType.mult
    ADD = mybir.AluOpType.add
    MAX = mybir.AluOpType.max
    Exp = mybir.ActivationFunctionType.Exp
    Copy = mybir.ActivationFunctionType.Copy

    def slot(j, g):
        return j * GL + g if g < GL else GRP * GL + j * GH + (g - GL)

    nc = bacc.Bacc(num_devices=CORES, num_swdge_queues=4)

    xT = nc.declare_dram_parameter("xT", [P, NPC], bf, isOutput=False)
    rhsA = nc.declare_dram_parameter("rhsA", [P, 136], bf, isOutput=False)
    b1row = nc.declare_dram_parameter("b1row", [2 * M, 136], f32, isOutput=False)
    rhsC = nc.declare_dram_parameter("rhsC", [P, 68], bf, isOutput=False)
    b2row = nc.declare_dram_parameter("b2row", [M, 68], f32, isOutput=False)
    iota = nc.declare_dram_parameter("iota", [P, P], bf, isOutput=False)
    ident = nc.declare_dram_parameter("ident", [P, P], bf, isOutput=False)
    NG = BPC // GRP
    ilo = nc.declare_dram_parameter("ilo", [NG, P, GRP * GL * 8], i16, isOutput=False)
    ihi = nc.declare_dram_parameter("ihi", [NG, P, max(GRP * GH * 8, 8)], i16, isOutput=False)
    dstp = nc.declare_dram_parameter("dstp", [NG, P, GRP * 2 * G], bf, isOutput=False)
    outp = nc.declare_dram_parameter("out", [NPC, D_OUT], f32, isOutput=True)

    if debug:
        dbg_ar1 = nc.declare_dram_parameter("dbg_ar1", [2, NPC], bf, isOutput=True)
        dbg_adb = nc.declare_dram_parameter("dbg_adb", [P, 256], bf, isOutput=True)
        dbg_adv = nc.declare_dram_parameter("dbg_adv", [P, 2 * GRP * (GL + GH)], f32, isOutput=True)
        dbg_alpha = nc.declare_dram_parameter("dbg_alpha", [P, 2 * GRP * (GL + GH)], f32, isOutput=True)
        dbg_w = nc.declare_dram_parameter("dbg_w", [P, 2 * GRP * (GL + GH)], f32, isOutput=True)
        dbg_oh = nc.declare_dram_parameter("dbg_oh", [P, GRP * (GL + GH) * 64], bf, isOutput=True)
        dbg_gt = nc.declare_dram_parameter("dbg_gt", [P, GRP * (GL + GH) * R1], bf, isOutput=True)
        dbg_ps = nc.declare_dram_parameter("dbg_ps", [M, 132], f32, isOutput=True)
        dbg_h = nc.declare_dram_parameter("dbg_h", [M, P], bf, isOutput=True)
        dbg_ar2 = nc.declare_dram_parameter("dbg_ar2", [1, NPC], bf, isOutput=True)

    t1l = nc.dram_tensor("t1l", [NPC, R1], bf)
    t1 = nc.dram_tensor("t1", [NP, R1], bf, addr_space="Shared")
    t2l = nc.dram_tensor("t2l", [NPC, R2], bf)
    t2 = nc.dram_tensor("t2", [NP, R2], bf, addr_space="Shared")

    groups = [list(range(CORES))]

    QN = [0]

    def next_q():
        QN[0] = (QN[0] + 1) % 4
        return QN[0]

    def chunked_gather(out_tile, slot0, tab_ap, idx_tile, nslots, elem):
        CH = 8  # 1024 indices per dma_gather (single-packet 64-desc limit)
        done = 0
        while done < nslots:
            take = min(CH, nslots - done)
            n = take * P
            nc.gpsimd.dma_gather(
                _ap(out_tile, (slot0 + done) * elem, [[elem, take], [1, elem]]),
                tab_ap,
                idx_tile[:, done * 8 : (done + take) * 8],
                n, n, elem, queue_num=next_q(),
            )
            done += take

    with TileContext(nc) as tc:
        with tc.tile_pool(name="const", bufs=1) as cp:
            rhsA_sb = cp.tile([P, 136], bf)
            nc.sync.dma_start(out=rhsA_sb[:], in_=rhsA[:])
            b1row_sb = cp.tile([2 * M, 136], f32)
            nc.sync.dma_start(out=b1row_sb[:], in_=b1row[:])
            rhsC_sb = cp.tile([P, 68], bf)
            nc.sync.dma_start(out=rhsC_sb[:], in_=rhsC[:])
            b2row_sb = cp.tile([M, 68], f32)
            nc.sync.dma_start(out=b2row_sb[:], in_=b2row[:])
            iota_sb = cp.tile([P, P], bf)
            nc.sync.dma_start(out=iota_sb[:], in_=iota[:])
            ident_sb = cp.tile([P, P], bf)
            nc.sync.dma_start(out=ident_sb[:], in_=ident[:])
            zero_sb = cp.tile([M, P], f32)
            nc.vector.memset(zero_sb[:], 0.0)
            adb1 = cp.tile([P, 2 * NPC], bf)     # a_dst layer1: head h at cols h*NPC + n
            adb2 = cp.tile([P, NPC], bf)         # a_dst layer2
            ar2 = cp.tile([1, NPC], bf)

            # ---------------- phase A: layer-1 table rows for own nodes
            with (
                tc.tile_pool(name="sbA0", bufs=1) as pa0,
                tc.tile_pool(name="sbA", bufs=3) as pa,
                tc.tile_pool(name="psA", bufs=2, space="PSUM") as ppa,
            ):
                xT_sb = pa0.tile([P, NPC], bf)
                nc.sync.dma_start(out=xT_sb[:], in_=xT[:])
                ar1_h0 = pa0.tile([1, NPC], bf)
                ar1_h1 = pa0.tile([1, NPC], bf)
                ag1_chunks = {}
                for k in range(len(CH_LEN)):
                    eb = (CH_CST[k] + CH_LEN[k]) // M - 1
                    ag1_chunks[eb - (eb % 2)] = k
                for b in range(0, BPC, 2):
                    ps = ppa.tile([2 * M, 136], f32, tag="psA")
                    nc.tensor.matmul(
                        ps[:], lhsT=xT_sb[:, b * M : (b + 2) * M], rhs=rhsA_sb[:],
                        start=True, stop=True,
                    )
                    row = pa.tile([2 * M, R1], bf, tag="rowA")
                    nc.vector.tensor_tensor(out=row[:, 0:136], in0=ps[:], in1=b1row_sb[:], op=ADD)
                    nc.sync.dma_start(out=t1l[b * M : (b + 2) * M, :], in_=row[:])
                    if b in ag1_chunks:
                        k = ag1_chunks[b]
                        r0 = CH_CST[k]
                        g0 = CH_GST[k]
                        ln = CH_LEN[k]
                        nc.gpsimd.collective_compute(
                            "AllGather", mybir.AluOpType.bypass, replica_groups=groups,
                            ins=[t1l[r0 : r0 + ln, :]],
                            outs=[t1[g0 : g0 + CORES * ln, :]],
                        )
                # a_dst rows on partition 0: 512-node chunks per head
                off = 0
                while off < NPC:
                    w_ = min(512, NPC - off)
                    for hh, art in ((0, ar1_h0), (1, ar1_h1)):
                        psr = ppa.tile([1, 512], f32, tag=f"psr{hh}")
                        nc.tensor.matmul(
                            psr[:, 0:w_], lhsT=rhsA_sb[:, 134 + hh : 135 + hh],
                            rhs=xT_sb[:, off : off + w_], start=True, stop=True,
                        )
                        nc.vector.tensor_copy(out=art[:, off : off + w_], in_=psr[:, 0:w_])
                    off += w_

                nc.gpsimd.partition_broadcast(adb1[:, 0:NPC], ar1_h0[:], P)
                nc.gpsimd.partition_broadcast(adb1[:, NPC : 2 * NPC], ar1_h1[:], P)
                if debug:
                    nc.sync.dma_start(out=dbg_ar1[0:1, :], in_=ar1_h0[:])
                    nc.sync.dma_start(out=dbg_ar1[1:2, :], in_=ar1_h1[:])
                    nc.sync.dma_start(out=dbg_adb[:, 0:128], in_=adb1[:, 0:128])
                    nc.sync.dma_start(out=dbg_adb[:, 128:256], in_=adb1[:, NPC : NPC + 128])


            # ---------------- phase B: layer-1 edges -> h -> table2 rows
            with (
                tc.tile_pool(name="sbB", bufs=3) as pb,
                tc.tile_pool(name="psB", bufs=2, space="PSUM") as ppb,
                tc.tile_pool(name="psB1", bufs=1, space="PSUM") as ppb1,
            ):
                for s in range(NG):
                    li = pb.tile([P, GRP * GL * 8], i16, tag="li")
                    nc.sync.dma_start(out=li[:], in_=ilo[s])
                    hi_t = pb.tile([P, max(GRP * GH * 8, 8)], i16, tag="hi")
                    nc.sync.dma_start(out=hi_t[:], in_=ihi[s])
                    dp = pb.tile([P, GRP * 2 * G], bf, tag="dp")
                    nc.sync.dma_start(out=dp[:], in_=dstp[s])

                    gt = pb.tile([P, T * R1], bf, tag="gt")
                    chunked_gather(gt, 0, t1[:], li, GRP * GL, R1)
                    if GH:
                        chunked_gather(gt, GRP * GL, t1[HALF:, :], hi_t, GRP * GH, R1)

                    # onehot, SLOT-major single copy: cols slot*64 + m
                    oh = pb.tile([P, T * 64], bf, tag="oh")
                    nc.vector.tensor_tensor(
                        out=_ap(oh, 0, [[64, T], [2, 32], [1, 2]]),
                        in0=_ap(iota_sb, 0, [[0, T], [2, 32], [1, 2]]),
                        in1=_ap(dp, 0, [[2, T], [0, 32], [1, 2]]),
                        op=EQ,
                    )
                    # per-edge a_dst: tmp[slot,hh,m] = oh[slot,m] * adb1[hh,node],
                    # then one reduce over m for all slots
                    tmp = pb.tile([P, T * P], bf, tag="tmp")
                    for j in range(GRP):
                        nd = (s * GRP + j) * M
                        nc.vector.tensor_tensor(
                            out=_ap(tmp, j * GL * P, [[P, GL], [64, 2], [1, 64]]),
                            in0=_ap(oh, j * GL * 64, [[64, GL], [0, 2], [1, 64]]),
                            in1=_ap(adb1, nd, [[0, GL], [NPC, 2], [1, 64]]),
                            op=MUL,
                        )
                        if GH:
                            nc.vector.tensor_tensor(
                                out=_ap(tmp, (GRP * GL + j * GH) * P, [[P, GH], [64, 2], [1, 64]]),
                                in0=_ap(oh, (GRP * GL + j * GH) * 64, [[64, GH], [0, 2], [1, 64]]),
                                in1=_ap(adb1, nd, [[0, GH], [NPC, 2], [1, 64]]),
                                op=MUL,
                            )
                    nc.vector.tensor_tensor(
                        out=_ap(tmp, 0, [[P, T], [64, 2], [1, 32]]),
                        in0=_ap(tmp, 0, [[P, T], [64, 2], [1, 32]]),
                        in1=_ap(tmp, 32, [[P, T], [64, 2], [1, 32]]),
                        op=ADD,
                    )
                    adv = pb.tile([P, T * 2], f32, tag="adv")
                    nc.vector.tensor_reduce(
                        out=adv[:],
                        in_=_ap(tmp, 0, [[P, T], [64, 2], [1, 32]]),
                        axis=mybir.AxisListType.X, op=ADD,
                    )
                    # alpha = a_src[src] + a_dst[dst]; w = exp(lrelu(alpha))
                    alpha = pb.tile([P, T * 2], f32, tag="alpha")
                    nc.vector.tensor_tensor(
                        out=alpha[:],
                        in0=_ap(gt, 132, [[R1, T], [1, 2]]),
                        in1=adv[:], op=ADD,
                    )
                    lr = pb.tile([P, T * 2], f32, tag="lr")
                    nc.vector.scalar_tensor_tensor(
                        out=lr[:], in0=alpha[:], scalar=SLOPE, in1=alpha[:],
                        op0=MUL, op1=MAX,
                    )
                    w = pb.tile([P, T * 2], f32, tag="w")
                    nc.scalar.activation(w[:], lr[:], Exp)
                    if debug and s == 0:
                        nc.sync.dma_start(out=dbg_adv[:], in_=adv[:])
                        nc.sync.dma_start(out=dbg_alpha[:], in_=alpha[:])
                        nc.sync.dma_start(out=dbg_w[:], in_=w[:])
                        nc.sync.dma_start(out=dbg_oh[:], in_=oh[:])
                        nc.sync.dma_start(out=dbg_gt[:], in_=gt[:])

                    # ow = oh * w (stride-0 broadcast of w over the 64 m cols);
                    # reuses tmp's buffer (tmp is dead after the reduce)
                    ow = tmp
                    nc.vector.tensor_tensor(
                        out=_ap(ow, 0, [[P, T], [64, 2], [1, 64]]),
                        in0=_ap(oh, 0, [[64, T], [0, 2], [1, 64]]),
                        in1=_ap(w, 0, [[2, T], [1, 2], [0, 64]]),
                        op=MUL,
                    )

                    for j in range(GRP):
                        blk = s * GRP + j
                        ps0 = ppb.tile([M, 66], f32, tag="psB0")
                        ps1 = ppb.tile([M, 66], f32, tag="psB1")
                        for g in range(G):
                            base = slot(j, g) * R1
                            cb = slot(j, g) * P
                            nc.tensor.matmul(
                                ps0[:],
                                lhsT=ow[:, cb : cb + 64],
                                rhs=gt[:, base : base + 66],
                                start=(g == 0), stop=(g == G - 1),
                            )
                            nc.tensor.matmul(
                                ps1[:],
                                lhsT=ow[:, cb + 64 : cb + 128],
                                rhs=gt[:, base + 66 : base + 132],
                                start=(g == 0), stop=(g == G - 1),
                            )
                        zs = pb.tile([M, 2], f32, tag="zs")
                        nc.vector.tensor_scalar(zs[:, 0:1], ps0[:, 64:65], 1e-30, None, MAX)
                        nc.vector.tensor_scalar(zs[:, 1:2], ps1[:, 64:65], 1e-30, None, MAX)
                        rz = pb.tile([M, 2], f32, tag="rz")
                        nc.vector.reciprocal(rz[:], zs[:])
                        h = pb.tile([M, P], bf, tag="h")
                        nc.vector.scalar_tensor_tensor(
                            out=h[:, 0:64], in0=ps0[:, 0:64], scalar=rz[:, 0:1],
                            in1=zero_sb[:, 0:64], op0=MUL, op1=MAX,
                        )
                        nc.vector.scalar_tensor_tensor(
                            out=h[:, 64:128], in0=ps1[:, 0:64], scalar=rz[:, 1:2],
                            in1=zero_sb[:, 64:128], op0=MUL, op1=MAX,
                        )
                        if debug and s == 0 and j == 0:
                            pscopy = pb.tile([M, 132], f32, tag="pscopy")
                            nc.vector.tensor_copy(out=pscopy[:, 0:66], in_=ps0[:])
                            nc.vector.tensor_copy(out=pscopy[:, 66:132], in_=ps1[:])
                            nc.sync.dma_start(out=dbg_ps[:], in_=pscopy[:])
                            nc.sync.dma_start(out=dbg_h[:], in_=h[:])
                        pt = ppb1.tile([P, M], bf, tag="psT")
                        nc.tensor.transpose(out=pt[:], in_=h[:], identity=ident_sb[:64, :64])
                        hT = pb.tile([P, M], bf, tag="hT")
                        nc.scalar.activation(hT[:], pt[:], Copy)
                        psc = ppb.tile([M, 68], f32, tag="psC")
                        nc.tensor.matmul(psc[:], lhsT=hT[:], rhs=rhsC_sb[:], start=True, stop=True)
                        row2 = pb.tile([M, R2], bf, tag="row2")
                        nc.vector.tensor_tensor(out=row2[:, 0:68], in0=psc[:], in1=b2row_sb[:], op=ADD)
                        nc.scalar.dma_start(out=t2l[blk * M : (blk + 1) * M, :], in_=row2[:])
                        # layer-2 a_dst row chunk (nodes of this block on partition 0)
                        psd = ppb1.tile([1, M], f32, tag="psD2")
                        nc.tensor.matmul(psd[:], lhsT=rhsC_sb[:, 67:68], rhs=hT[:, 0:M],
                                         start=True, stop=True)
                        nc.scalar.activation(ar2[:, blk * M : (blk + 1) * M], psd[:], Copy)
                    ag2_chunks = {
                        ((CH_CST[k] + CH_LEN[k]) // M - 1) // GRP: k
                        for k in range(len(CH_LEN))
                    }
                    if s in ag2_chunks:
                        k = ag2_chunks[s]
                        r0 = CH_CST[k]
                        g0 = CH_GST[k]
                        ln = CH_LEN[k]
                        nc.gpsimd.collective_compute(
                            "AllGather", mybir.AluOpType.bypass, replica_groups=groups,
                            ins=[t2l[r0 : r0 + ln, :]],
                            outs=[t2[g0 : g0 + CORES * ln, :]],
                        )

            nc.gpsimd.partition_broadcast(adb2[:], ar2[:], P)
            if debug:
                nc.sync.dma_start(out=dbg_ar2[:], in_=ar2[:])


            # ---------------- phase D: layer-2 edges -> output
            with (
                tc.tile_pool(name="sbD", bufs=4) as pd,
                tc.tile_pool(name="psD", bufs=2, space="PSUM") as ppd,
            ):
                for s in range(NG):
                    li = pd.tile([P, GRP * GL * 8], i16, tag="li2")
                    nc.sync.dma_start(out=li[:], in_=ilo[s])
                    hi_t = pd.tile([P, max(GRP * GH * 8, 8)], i16, tag="hi2")
                    nc.sync.dma_start(out=hi_t[:], in_=ihi[s])
                    dp = pd.tile([P, GRP * 2 * G], bf, tag="dp2")
                    nc.sync.dma_start(out=dp[:], in_=dstp[s])

                    gt2 = pd.tile([P, T * R2], bf, tag="gt2")
                    chunked_gather(gt2, 0, t2[:], li, GRP * GL, R2)
                    if GH:
                        chunked_gather(gt2, GRP * GL, t2[HALF:, :], hi_t, GRP * GH, R2)

                    oh = pd.tile([P, T * 64], bf, tag="oh2")
                    nc.vector.tensor_tensor(
                        out=_ap(oh, 0, [[64, T], [2, 32], [1, 2]]),
                        in0=_ap(iota_sb, 0, [[0, T], [2, 32], [1, 2]]),
                        in1=_ap(dp, 0, [[2, T], [0, 32], [1, 2]]),
                        op=EQ,
                    )
                    tmp = pd.tile([P, T * 64], bf, tag="tmp2")
                    for j in range(GRP):
                        nd = (s * GRP + j) * M
                        nc.vector.tensor_tensor(
                            out=_ap(tmp, j * GL * 64, [[64, GL], [1, 64]]),
                            in0=_ap(oh, j * GL * 64, [[64, GL], [1, 64]]),
                            in1=_ap(adb2, nd, [[0, GL], [1, 64]]),
                            op=MUL,
                        )
                        if GH:
                            hb = (GRP * GL + j * GH) * 64
                            nc.vector.tensor_tensor(
                                out=_ap(tmp, hb, [[64, GH], [1, 64]]),
                                in0=_ap(oh, hb, [[64, GH], [1, 64]]),
                                in1=_ap(adb2, nd, [[0, GH], [1, 64]]),
                                op=MUL,
                            )
                    nc.vector.tensor_tensor(
                        out=_ap(tmp, 0, [[64, T], [1, 32]]),
                        in0=_ap(tmp, 0, [[64, T], [1, 32]]),
                        in1=_ap(tmp, 32, [[64, T], [1, 32]]),
                        op=ADD,
                    )
                    adv = pd.tile([P, T], f32, tag="adv2")
                    nc.vector.tensor_reduce(
                        out=adv[:],
                        in_=_ap(tmp, 0, [[64, T], [1, 32]]),
                        axis=mybir.AxisListType.X, op=ADD,
                    )
                    alpha = pd.tile([P, T], f32, tag="alpha2")
                    nc.vector.tensor_tensor(
                        out=alpha[:],
                        in0=_ap(gt2, 66, [[R2, T]]),
                        in1=adv[:], op=ADD,
                    )
                    lr = pd.tile([P, T], f32, tag="lr2")
                    nc.vector.scalar_tensor_tensor(
                        out=lr[:], in0=alpha[:], scalar=SLOPE, in1=alpha[:],
                        op0=MUL, op1=MAX,
                    )
                    w = pd.tile([P, T], f32, tag="w2")
                    nc.scalar.activation(w[:], lr[:], Exp)

                    ow = tmp
                    nc.vector.tensor_tensor(
                        out=_ap(ow, 0, [[64, T], [1, 64]]),
                        in0=_ap(oh, 0, [[64, T], [1, 64]]),
                        in1=_ap(w, 0, [[1, T], [0, 64]]),
                        op=MUL,
                    )

                    for j in range(GRP):
                        blk = s * GRP + j
                        ps = ppd.tile([M, 66], f32, tag="psD")
                        for g in range(G):
                            base = slot(j, g) * R2
                            cb = slot(j, g) * 64
                            nc.tensor.matmul(
                                ps[:],
                                lhsT=ow[:, cb : cb + 64],
                                rhs=gt2[:, base : base + 66],
                                start=(g == 0), stop=(g == G - 1),
                            )
                        zs = pd.tile([M, 1], f32, tag="zs2")
                        nc.vector.tensor_scalar(zs[:], ps[:, 64:65], 1e-30, None, MAX)
                        rz = pd.tile([M, 1], f32, tag="rz2")
                        nc.vector.reciprocal(rz[:], zs[:])
                        o2 = pd.tile([M, D_OUT], f32, tag="o2")
                        nc.vector.scalar_tensor_tensor(
                            out=o2[:], in0=ps[:, 0:64], scalar=rz[:, 0:1],
                            in1=zero_sb[:, 0:64], op0=MUL, op1=ADD,
                        )
                        nc.scalar.dma_start(out=outp[blk * M : (blk + 1) * M, :], in_=o2[:])

    nc.finalize()
    return nc


# ------------------------------------------------------------------ driver

_CACHE = {}


def kernel(x, edge_index, W1, att_src1, att_dst1, b1, W2, att_src2, att_dst2, b2):
    GL, GH, per_core, perm = _edge_buckets(edge_index)
    consts = _host_consts(x, W1, att_src1, att_dst1, b1, W2, att_src2, att_dst2, b2, perm)

    if (GL, GH) not in _CACHE:
        _CACHE[(GL, GH)] = build_program(GL, GH)
    nc = _CACHE[(GL, GH)]

    in_maps = []
    for c in range(CORES):
        m = dict(consts)
        m["xT"] = np.ascontiguousarray(consts["xT"][:, c * NPC : (c + 1) * NPC])
        m.update(per_core[c])
        in_maps.append(m)

    res = run_bass_kernel_spmd(nc, in_maps, list(range(CORES)))
    out = np.concatenate([np.asarray(res.results[c]["out"]) for c in range(CORES)], axis=0)
    return np.ascontiguousarray(out[perm]).astype(np.float32)
